# revision 1
# baseline (speedup 1.0000x reference)
"""Bayesian linear layer (reparameterized per-sample weights) on 8 trn2 NeuronCores.

y[b,o] = sum_i x[b,i] * (mu[o,i] + softplus(rho[o,i]) * eps_w[b,o,i])
         + bias_mu[o] + softplus(bias_rho[o]) * eps_b[b,o]

Sharding: data-parallel over batch. 8 cores x 32 samples. mu/rho replicated.

Per-core pipeline (the 128 MB eps_w shard dominates: ~360 GB/s HBM -> ~370 us):
  1. DMA eps_w[b] natural tiles [o=128p, i free] (contiguous, full BW)
  2. PE transpose 128x128 chunks -> PSUM   (gets i onto partitions)
  3. DVE single pass u = epsT (*) sigmaT, PSUM -> SBUF
  4. PE reduce-matmul, stationary = x[b, i_chunk] column (m=1), moving = u,
     float32r so fp32 data streams at 1 cycle/row; accumulates y2[b,:] in PSUM
  5. y_mu + bias terms precomputed into C[32,1024]; per-sample rows added via
     SBUF->SBUF accumulating DMA; one store of C to HBM.
"""

import numpy as np

import concourse.bass as bass
from concourse import bacc
import concourse.mybir as mybir
import concourse.tile as tile
from concourse.bass import ts
from concourse.bass_utils import run_bass_kernel_spmd
from concourse.masks import make_identity

FP32 = mybir.dt.float32
F32R = mybir.dt.float32r
AF = mybir.ActivationFunctionType

F = 1024          # feature dim (in == out)
N_CORES = 8
B_FULL = 256
NCH = F // 128    # 8 chunks of 128


def build_nc(BL: int, eps_bufs=3, pt_bufs=2, y2_bufs=2, u_bufs=4,
             ablate=()) -> bass.Bass:
    """Build the per-core Bass program for a local batch of BL samples."""
    nc = bacc.Bacc(None, target_bir_lowering=False)

    x_d = nc.declare_dram_parameter("x", [BL, F], FP32, isOutput=False)
    mu_d = nc.declare_dram_parameter("weight_mu", [F, F], FP32, isOutput=False)
    rho_d = nc.declare_dram_parameter("weight_rho", [F, F], FP32, isOutput=False)
    bmu_d = nc.declare_dram_parameter("bias_mu", [F], FP32, isOutput=False)
    brho_d = nc.declare_dram_parameter("bias_rho", [F], FP32, isOutput=False)
    epsw_d = nc.declare_dram_parameter("eps_w", [BL, F, F], FP32, isOutput=False)
    epsb_d = nc.declare_dram_parameter("eps_b", [BL, F], FP32, isOutput=False)
    y_d = nc.declare_dram_parameter("y", [BL, F], FP32, isOutput=True)

    # [b, (c p), i] -> [b, p, c, i]: partition p indexes o within chunk c
    epsw_t = epsw_d[:].rearrange("b (c p) i -> b p c i", p=128)
    mu_t = mu_d[:].rearrange("(c p) i -> p c i", p=128)
    rho_t = rho_d[:].rearrange("(c p) i -> p c i", p=128)

    with tile.TileContext(nc) as tc:
        with (
            tc.tile_pool(name="persist", bufs=1) as persist,
            tc.tile_pool(name="eps", bufs=eps_bufs) as epsp,
            tc.tile_pool(name="u", bufs=u_bufs) as up,
            tc.tile_pool(name="yrow", bufs=4) as yrowp,
            tc.tile_pool(name="pt", bufs=pt_bufs, space="PSUM") as ptp,
            tc.tile_pool(name="py2", bufs=y2_bufs, space="PSUM") as py2p,
        ):
            # ---------------- setup ----------------
            ident = persist.tile([128, 128], FP32)
            make_identity(nc, ident)

            # sigmaT[i, o] = softplus(rho[o, i]).T ; layout [128p(i in chunk k), k, o]
            sigT = persist.tile([128, NCH, F], FP32)
            rho_s = epsp.tile([128, NCH, F], FP32, tag="epst")
            nc.sync.dma_start(out=rho_s, in_=rho_t)
            for k in range(NCH):
                pt_k = ptp.tile([128, F], FP32, tag="pt_k")
                for c in range(NCH):
                    nc.tensor.transpose(
                        out=pt_k[:, ts(c, 128)],
                        in_=rho_s[:, c, ts(k, 128)],
                        identity=ident,
                    )
                sp_tmp = up.tile([128, F], FP32, tag="sp_tmp", name="sp_tmp")
                nc.scalar.activation(out=sp_tmp, in_=pt_k, func=AF.Exp)
                # softplus(x) = ln(1 + exp(x)); rho <= ~0 so no overflow
                nc.scalar.activation(out=sigT[:, k, :], in_=sp_tmp, func=AF.Ln, bias=1.0)

            tc.strict_bb_all_engine_barrier()

            # muT (setup only; slot returns to the eps pool afterwards)
            muT = epsp.tile([128, NCH, F], FP32, tag="epst")
            mu_s = epsp.tile([128, NCH, F], FP32, tag="epst")
            nc.sync.dma_start(out=mu_s, in_=mu_t)
            for k in range(NCH):
                pt_k = ptp.tile([128, F], FP32, tag="pt_k")
                for c in range(NCH):
                    nc.tensor.transpose(
                        out=pt_k[:, ts(c, 128)],
                        in_=mu_s[:, c, ts(k, 128)],
                        identity=ident,
                    )
                nc.scalar.copy(out=muT[:, k, :], in_=pt_k)

            tc.strict_bb_all_engine_barrier()

            # xT[i, b] ; layout [128p(i in chunk k), k, b]
            x_nat = persist.tile([BL, F], FP32)
            nc.sync.dma_start(out=x_nat, in_=x_d[:])
            xT = persist.tile([128, NCH, BL], FP32)
            xTr = persist.tile([128, NCH, BL], F32R)
            for k in range(NCH):
                ptx = ptp.tile([128, BL], FP32, tag="pt_k")
                nc.tensor.transpose(
                    out=ptx,
                    in_=x_nat[:, ts(k, 128)],
                    identity=ident[:BL, :BL],
                )
                nc.scalar.copy(out=xT[:, k, :], in_=ptx)
                nc.vector.tensor_copy(xTr[:, k, :], xT[:, k, :])

            tc.strict_bb_all_engine_barrier()

            # y_mu[b, o] = sum_i x[b,i] mu[o,i]  (full fp32 precision)
            ymu_ps = []
            for h in range(2):
                yp = ptp.tile([BL, 512], FP32, tag="pt_k")
                for k in range(NCH):
                    nc.tensor.matmul(
                        out=yp,
                        lhsT=xT[:, k, :],
                        rhs=muT[:, k, ts(h, 512)],
                        start=(k == 0),
                        stop=(k == NCH - 1),
                    )
                ymu_ps.append(yp)

            # C[b, o] = y_mu + bias_mu + softplus(bias_rho) * eps_b
            bmu_b = persist.tile([BL, F], FP32)
            nc.gpsimd.dma_start(
                out=bmu_b,
                in_=bass.AP(tensor=bmu_d, offset=0, ap=[[0, BL], [1, F]]),
            )
            sb_b = persist.tile([BL, F], FP32)
            nc.gpsimd.dma_start(
                out=sb_b,
                in_=bass.AP(tensor=brho_d, offset=0, ap=[[0, BL], [1, F]]),
            )
            nc.scalar.activation(out=sb_b, in_=sb_b, func=AF.Exp)
            nc.scalar.activation(out=sb_b, in_=sb_b, func=AF.Ln, bias=1.0)
            epsb_s = persist.tile([BL, F], FP32)
            nc.sync.dma_start(out=epsb_s, in_=epsb_d[:])

            C = persist.tile([BL, F], FP32)
            nc.vector.tensor_mul(C, sb_b, epsb_s)
            nc.vector.tensor_add(C, C, bmu_b)
            for h in range(2):
                nc.vector.tensor_add(C[:, ts(h, 512)], C[:, ts(h, 512)], ymu_ps[h])

            tc.strict_bb_all_engine_barrier()

            # ---------------- main loop over samples ----------------
            for b in range(BL):
                eb = epsp.tile([128, NCH, F], FP32, tag="epst")
                nc.sync.dma_start(out=eb, in_=epsw_t[b])

                y2 = [
                    py2p.tile([1, 512], FP32, tag=f"y2_{h}", name=f"y2_{h}")
                    for h in range(2)
                ]
                for k in range(NCH):
                    if "notrans" in ablate:
                        break
                    pt_k = ptp.tile([128, F], FP32, tag="pt_k")
                    for c in range(NCH):
                        nc.tensor.transpose(
                            out=pt_k[:, ts(c, 128)],
                            in_=eb[:, c, ts(k, 128)],
                            identity=ident,
                        )
                    if "nott" in ablate:
                        continue
                    u_k = up.tile([128, F], F32R)
                    nc.vector.tensor_mul(u_k, pt_k, sigT[:, k, :])
                    if "nomm" in ablate:
                        continue
                    for h in range(2):
                        nc.tensor.matmul(
                            out=y2[h],
                            lhsT=xTr[:, k, b : b + 1],
                            rhs=u_k[:, ts(h, 512)],
                            start=(k == 0),
                            stop=(k == NCH - 1),
                        )

                if not ablate:
                    yrow = yrowp.tile([1, F], FP32)
                    for h in range(2):
                        nc.scalar.copy(out=yrow[:, ts(h, 512)], in_=y2[h])
                    nc.gpsimd.dma_start(
                        out=C[b : b + 1, :], in_=yrow, accum_op=mybir.AluOpType.add
                    )

            nc.sync.dma_start(out=y_d[:], in_=C)

    nc.compile()
    return nc


_NC_CACHE: dict[int, bass.Bass] = {}


def _get_nc(BL: int) -> bass.Bass:
    if BL not in _NC_CACHE:
        _NC_CACHE[BL] = build_nc(BL)
    return _NC_CACHE[BL]


def kernel(x, weight_mu, weight_rho, bias_mu, bias_rho, eps_w, eps_b):
    B = x.shape[0]
    BL = B // N_CORES
    nc = _get_nc(BL)

    x = np.ascontiguousarray(np.asarray(x, dtype=np.float32))
    weight_mu = np.ascontiguousarray(np.asarray(weight_mu, dtype=np.float32))
    weight_rho = np.ascontiguousarray(np.asarray(weight_rho, dtype=np.float32))
    bias_mu = np.ascontiguousarray(np.asarray(bias_mu, dtype=np.float32))
    bias_rho = np.ascontiguousarray(np.asarray(bias_rho, dtype=np.float32))
    eps_w = np.ascontiguousarray(np.asarray(eps_w, dtype=np.float32))
    eps_b = np.ascontiguousarray(np.asarray(eps_b, dtype=np.float32))

    in_maps = []
    for i in range(N_CORES):
        sl = slice(i * BL, (i + 1) * BL)
        in_maps.append(
            {
                "x": x[sl],
                "weight_mu": weight_mu,
                "weight_rho": weight_rho,
                "bias_mu": bias_mu,
                "bias_rho": bias_rho,
                "eps_w": eps_w[sl],
                "eps_b": eps_b[sl],
            }
        )

    res = run_bass_kernel_spmd(nc, in_maps, core_ids=list(range(N_CORES)))
    return np.concatenate([r["y"] for r in res.results], axis=0)



# revision 17
# speedup vs baseline: 1.1998x; 1.1998x over previous
"""Bayesian linear layer (reparameterized per-sample weights) on 8 trn2 NeuronCores.

y[b,o] = sum_i x[b,i] * (mu[o,i] + softplus(rho[o,i]) * eps_w[b,o,i])
         + bias_mu[o] + softplus(bias_rho[o]) * eps_b[b,o]

Sharding: data-parallel over batch. 8 cores x 32 samples. mu/rho replicated.

The 128 MiB fp32 eps_w shard dominates (~360 GB/s HBM -> ~375 us floor).
Per-core pipeline, all in NATURAL layout (o on partitions, no per-sample
transposes):
  1. SWDGE cast-DMA eps_w[b] -> bf16 SBUF, o split "(p c)" so each
     partition line is one contiguous 32 KiB read (8 rows).
  2. y2[b,o] = sum_i eps*sigma*x: DVE computes sx = sigma (*) bcast(x[b])
     (one 2x-rate bf16 tensor_tensor over [128, 8*1024]), then per o-chunk
     either a fused tensor_tensor_reduce (DVE) or tensor_tensor + ACT
     activation-accumulate (free-axis sum) -> Y2all[:, c*BL+b].
  3. x[b] broadcast across partitions via a K=1 PE matmul (ones stationary)
     into PSUM, ACT-copied to bf16 SBUF.
  4. mu/ymu/bias path (setup-class work, overlapped with the stream):
     PE transposes mu -> muT, ymu = xT @ muT, C = ymu + bias_mu +
     softplus(bias_rho)*eps_b in fp32.
  5. End: 8 PE transposes of Y2all -> PSUM, strided DVE adds into C
     (o = 8p + c), single store of C.
"""

import numpy as np

import concourse.bass as bass
from concourse import bacc
import concourse.mybir as mybir
import concourse.tile as tile
from concourse.bass import ts
from concourse.bass_utils import run_bass_kernel_spmd
from concourse.masks import make_identity

FP32 = mybir.dt.float32
BF16 = mybir.dt.bfloat16
AF = mybir.ActivationFunctionType
ALU = mybir.AluOpType

F = 1024          # feature dim (in == out)
N_CORES = 8
B_FULL = 256
NCH = F // 128    # 8 o-chunks of 128 partitions

N_DVR = 1         # o-chunks reduced on DVE via tensor_reduce; rest on ACT accum
                  # (tensor_tensor_reduce faults on this HW runtime - do not use)


def _repeat_mid(ap: bass.AP, n: int) -> bass.AP:
    """[P, F] -> [P, n, F] with the middle dim stride-0 (free-axis repeat)."""
    assert len(ap.ap) == 2, ap.ap
    return bass.AP(tensor=ap.tensor, offset=ap.offset, ap=[ap.ap[0], [0, n], ap.ap[1]])


def build_nc(BL: int, eps_bufs=3, n_dvr=N_DVR) -> bass.Bass:
    nc = bacc.Bacc(None, target_bir_lowering=False)

    x_d = nc.declare_dram_parameter("x", [BL, F], FP32, isOutput=False)
    mu_d = nc.declare_dram_parameter("weight_mu", [F, F], FP32, isOutput=False)
    rho_d = nc.declare_dram_parameter("weight_rho", [F, F], FP32, isOutput=False)
    bmu_d = nc.declare_dram_parameter("bias_mu", [F], FP32, isOutput=False)
    brho_d = nc.declare_dram_parameter("bias_rho", [F], FP32, isOutput=False)
    epsw_d = nc.declare_dram_parameter("eps_w", [BL, F, F], FP32, isOutput=False)
    epsb_d = nc.declare_dram_parameter("eps_b", [BL, F], FP32, isOutput=False)
    y_d = nc.declare_dram_parameter("y", [BL, F], FP32, isOutput=True)

    # o = 8p + c: each partition line is 8 contiguous rows = 32 KiB per descriptor
    epsw_t = epsw_d[:].rearrange("b (p c) i -> b p c i", c=NCH)
    rho_t = rho_d[:].rearrange("(p c) i -> p c i", c=NCH)
    # mu keeps natural 128-blocks (o = 128c + p) so ymu columns come out natural
    mu_t = mu_d[:].rearrange("(c p) i -> p c i", p=128)

    with tile.TileContext(nc) as tc:
        with (
            tc.tile_pool(name="persist", bufs=1) as persist,
            tc.tile_pool(name="setup", bufs=1) as setupp,
            tc.tile_pool(name="eps", bufs=eps_bufs) as epsp,
            tc.tile_pool(name="sx", bufs=2) as sxp,
            tc.tile_pool(name="xb", bufs=3) as xbp,
            tc.tile_pool(name="tb", bufs=3) as tbp,
            tc.tile_pool(name="scr", bufs=2) as scrp,
            tc.tile_pool(name="pxb", bufs=2, space="PSUM") as pxbp,
            tc.tile_pool(name="pt", bufs=1, space="PSUM") as ptp,
            tc.tile_pool(name="pymu", bufs=1, space="PSUM") as pymup,
        ):
            # ---------------- setup ----------------
            ident = persist.tile([128, 128], FP32)
            make_identity(nc, ident)
            ones_bf = persist.tile([1, 128], BF16)
            nc.vector.memset(ones_bf, 1.0)

            # x natural (fp32, for the xT/ymu path) + bf16 copy for the bcast
            x_s = persist.tile([BL, F], FP32)
            nc.sync.dma_start(out=x_s, in_=x_d[:])
            x_bf = persist.tile([BL, F], BF16)
            nc.vector.tensor_copy(x_bf, x_s)

            # sigma_nat = softplus(rho) in (p c) layout, bf16
            sig = persist.tile([128, NCH, F], BF16)
            rho_s = setupp.tile([128, NCH, F], FP32, tag="setup_big")
            nc.sync.dma_start(out=rho_s, in_=rho_t)
            # softplus(x) = ln(1 + exp(x)); rho <= ~0 so no overflow
            nc.scalar.activation(out=rho_s, in_=rho_s, func=AF.Exp)
            nc.scalar.activation(out=sig, in_=rho_s, func=AF.Ln, bias=1.0)

            # Y2all[p, c*BL + b] = y2[b, o=8p+c]
            y2all = persist.tile([128, NCH * BL], FP32)
            # C[b, o] accumulates ymu + bias terms (+ y2 at the end)
            C = persist.tile([BL, F], FP32)

            # ---------------- main loop over samples ----------------
            for b in range(BL):
                eb = epsp.tile([128, NCH, F], BF16, tag="eps")
                nc.gpsimd.dma_start(out=eb, in_=epsw_t[b])

                # broadcast x[b] across 128 partitions: stage row b to
                # partition 0 (tiny DMA), ones^T @ stage on PE, ACT copy
                stage = xbp.tile([1, F], BF16, tag="stage")
                nc.sync.dma_start(out=stage, in_=x_bf[b : b + 1, :])
                xps = pxbp.tile([128, F], FP32, tag="xps")
                for h in range(2):
                    nc.tensor.matmul(
                        out=xps[:, ts(h, 512)],
                        lhsT=ones_bf,
                        rhs=stage[:, ts(h, 512)],
                        start=True,
                        stop=True,
                    )
                xb = xbp.tile([128, F], BF16, tag="xb")
                nc.scalar.copy(out=xb, in_=xps)

                # sx = sigma (*) x[b] broadcast over chunks (one 2x bf16 TT)
                sx = sxp.tile([128, NCH, F], BF16, tag="sx")
                nc.vector.tensor_mul(sx, sig, _repeat_mid(xb[:], NCH))

                for c in range(NCH):
                    idx = c * BL + b
                    tb = tbp.tile([128, F], BF16, tag="tb")
                    nc.vector.tensor_mul(tb, eb[:, c, :], sx[:, c, :])
                    if c < n_dvr:
                        nc.vector.tensor_reduce(
                            y2all[:, idx : idx + 1],
                            tb,
                            axis=mybir.AxisListType.X,
                            op=ALU.add,
                        )
                    else:
                        asc = scrp.tile([128, F], BF16, tag="ascr")
                        nc.scalar.activation(
                            out=asc,
                            in_=tb,
                            func=AF.Copy,
                            accum_out=y2all[:, idx : idx + 1],
                        )

            # ---------------- mu / ymu / bias path (overlaps the stream) ----
            mu_s = setupp.tile([128, NCH, F], FP32, tag="setup_big")
            nc.sync.dma_start(out=mu_s, in_=mu_t)
            muT = persist.tile([128, NCH, F], BF16)
            for k in range(NCH):
                pt_k = ptp.tile([128, F], FP32, tag="pt_k")
                for c in range(NCH):
                    nc.tensor.transpose(
                        out=pt_k[:, ts(c, 128)],
                        in_=mu_s[:, c, ts(k, 128)],
                        identity=ident,
                    )
                nc.scalar.copy(out=muT[:, k, :], in_=pt_k)

            # xT[i, b] bf16
            xT = persist.tile([128, NCH, BL], BF16)
            for k in range(NCH):
                ptx = ptp.tile([128, BL], FP32, tag="pt_k")
                nc.tensor.transpose(
                    out=ptx,
                    in_=x_s[:, ts(k, 128)],
                    identity=ident[:BL, :BL],
                )
                nc.scalar.copy(out=xT[:, k, :], in_=ptx)

            # ymu[b, o] = sum_i x[b,i] mu[o,i]
            ymu_ps = []
            for h in range(2):
                yp = pymup.tile([BL, 512], FP32, tag=f"ymu_{h}")
                for k in range(NCH):
                    nc.tensor.matmul(
                        out=yp,
                        lhsT=xT[:, k, :],
                        rhs=muT[:, k, ts(h, 512)],
                        start=(k == 0),
                        stop=(k == NCH - 1),
                    )
                ymu_ps.append(yp)

            # C = ymu + bias_mu + softplus(bias_rho) * eps_b
            bmu_b = persist.tile([BL, F], FP32)
            nc.gpsimd.dma_start(
                out=bmu_b,
                in_=bass.AP(tensor=bmu_d, offset=0, ap=[[0, BL], [1, F]]),
            )
            sb_b = persist.tile([BL, F], FP32)
            nc.gpsimd.dma_start(
                out=sb_b,
                in_=bass.AP(tensor=brho_d, offset=0, ap=[[0, BL], [1, F]]),
            )
            nc.scalar.activation(out=sb_b, in_=sb_b, func=AF.Exp)
            nc.scalar.activation(out=sb_b, in_=sb_b, func=AF.Ln, bias=1.0)
            epsb_s = persist.tile([BL, F], FP32)
            nc.sync.dma_start(out=epsb_s, in_=epsb_d[:])

            nc.vector.tensor_mul(C, sb_b, epsb_s)
            nc.vector.tensor_add(C, C, bmu_b)
            for h in range(2):
                nc.vector.tensor_add(C[:, ts(h, 512)], C[:, ts(h, 512)], ymu_ps[h])

            # ---------------- finish: C += y2 (transpose chunks), store ----
            C_r = C[:].rearrange("b (p c) -> b p c", c=NCH)
            for c in range(NCH):
                pty = ptp.tile([BL, 128], FP32, tag="pt_k")
                nc.tensor.transpose(
                    out=pty,
                    in_=y2all[:, c * BL : (c + 1) * BL],
                    identity=ident,
                )
                nc.vector.tensor_add(C_r[:, :, c], C_r[:, :, c], pty)

            nc.sync.dma_start(out=y_d[:], in_=C)

    nc.compile()
    return nc


_NC_CACHE: dict[int, bass.Bass] = {}


def _get_nc(BL: int) -> bass.Bass:
    if BL not in _NC_CACHE:
        _NC_CACHE[BL] = build_nc(BL)
    return _NC_CACHE[BL]


def kernel(x, weight_mu, weight_rho, bias_mu, bias_rho, eps_w, eps_b):
    B = x.shape[0]
    BL = B // N_CORES
    nc = _get_nc(BL)

    x = np.ascontiguousarray(np.asarray(x, dtype=np.float32))
    weight_mu = np.ascontiguousarray(np.asarray(weight_mu, dtype=np.float32))
    weight_rho = np.ascontiguousarray(np.asarray(weight_rho, dtype=np.float32))
    bias_mu = np.ascontiguousarray(np.asarray(bias_mu, dtype=np.float32))
    bias_rho = np.ascontiguousarray(np.asarray(bias_rho, dtype=np.float32))
    eps_w = np.ascontiguousarray(np.asarray(eps_w, dtype=np.float32))
    eps_b = np.ascontiguousarray(np.asarray(eps_b, dtype=np.float32))

    in_maps = []
    for i in range(N_CORES):
        sl = slice(i * BL, (i + 1) * BL)
        in_maps.append(
            {
                "x": x[sl],
                "weight_mu": weight_mu,
                "weight_rho": weight_rho,
                "bias_mu": bias_mu,
                "bias_rho": bias_rho,
                "eps_w": eps_w[sl],
                "eps_b": eps_b[sl],
            }
        )

    res = run_bass_kernel_spmd(nc, in_maps, core_ids=list(range(N_CORES)))
    return np.concatenate([r["y"] for r in res.results], axis=0)
